# revision 13
# baseline (speedup 1.0000x reference)
"""BitNet ternary linear (nn_BitNetLinear4Bit) Trainium2 Bass kernel.

out = x @ (alpha * clip(round(w/alpha), -1, 1))^T + bias
  x: [2, 2048, 4096] f32, w: [11008, 4096] f32, alpha: [1] f32, bias: [11008] f32
  -> out: [2, 2048, 11008] f32

Sharding: column-parallel over 8 cores. Each core gets the full x
(replicated) and a 1376-row slice of w / bias; it produces a
[4096, 1376] slice of the output which the host concatenates.

Per-core algorithm — mixed-precision K split to cut PE time:
  k < KBF (=2048): bf16 path. 16 normal bf16 matmuls per 128-token
    block per output group.
  k >= KBF: fp8e4 path at 2x rate via DoubleRow; w ternary values
    {-1,0,1} are exact in fp8. 8 DoubleRow matmuls of 256-k each.
    x is cast to e4m3 and XBAR-transposed as uint16 byte-pairs, then
    DVE de-interleaved into even/odd slot blocks (the ISA requires a
    block layout for the stationary operand). The moving (weight)
    operand reads the byte-interleaved pairs directly via stride-2
    APs.
  All transposes (x and w, both precisions) go through the DMA XBAR;
  the PE only runs matmuls. Work is emitted block-pipelined (MB=4
  token-blocks per block): weight-group prep and x-prep interleave
  so every strict-FIFO engine queue matches execution order, and
  compute for block b is emitted after prep for block b+... (2-block
  double buffering of transposed x tiles).
  Eviction: ACT copy*alpha, GpSimd +bias, out-DMA on the ACT HWDGE
  ring (keeps the sync ring free for loads/transposes).
  Expected rel err ~1.66e-2 (fp8 quantization of half of x),
  verified against an exact CPU model.

alpha is read on the host and baked into the program as an immediate;
the compiled program is cached keyed on alpha.
"""

import numpy as np

B, S, DIN, DOUT = 2, 2048, 4096, 11008
NCORES = 8
DOUT_SH = DOUT // NCORES  # 1376
TOK = B * S  # 4096
P = 128
KBF = 2048  # bf16 k range; the rest is fp8 DoubleRow
K8 = DIN - KBF
KO_BF = KBF // P  # 16
NB8 = K8 // (2 * P)  # 8 DoubleRow blocks of 256 k
MB = 2  # token-blocks per pipeline block
NBLK = TOK // P // MB  # 8


def _build(alpha_f, debug=False):
    import concourse.mybir as mybir
    from concourse import bacc
    from concourse.tile import TileContext

    f32 = mybir.dt.float32
    bf16 = mybir.dt.bfloat16
    f8 = mybir.dt.float8e4
    u16 = mybir.dt.uint16
    Alu = mybir.AluOpType
    Act = mybir.ActivationFunctionType
    DR = mybir.MatmulPerfMode.DoubleRow

    W_CHUNKS = (DOUT_SH + P - 1) // P  # 11 (last chunk 96 rows, zero-padded)
    QCOL = 2048
    NQ = DIN // QCOL  # 2 q-blocks; q=0 bf16, q=1 fp8
    GROUPS = []  # (first chunk, n chunks, dout start, real width)
    c = 0
    while c < W_CHUNKS:
        cc = min(4, W_CHUNKS - c)
        width = min(DOUT_SH, (c + cc) * P) - c * P
        GROUPS.append((c, cc, c * P, width))
        c += cc

    a2 = float(alpha_f) * 0.5

    nc = bacc.Bacc(None, target_bir_lowering=False, debug=debug)
    x_d = nc.dram_tensor("x", [TOK, DIN], f32, kind="ExternalInput")
    w_d = nc.dram_tensor("w", [DOUT_SH, DIN], f32, kind="ExternalInput")
    nc.dram_tensor("alpha", [1], f32, kind="ExternalInput")
    b_d = nc.dram_tensor("bias", [DOUT_SH], f32, kind="ExternalInput")
    o_d = nc.dram_tensor("out", [TOK, DOUT_SH], f32, kind="ExternalOutput")

    with TileContext(nc) as tc:
        with (
            tc.tile_pool(name="const", bufs=1) as const,
            tc.tile_pool(name="wres", bufs=1) as wres,
            tc.tile_pool(name="wq", bufs=2) as wq,
            tc.tile_pool(name="xp", bufs=2) as xp,
            tc.tile_pool(name="xcp", bufs=3) as xcp,
            tc.tile_pool(name="xtq", bufs=3) as xtq,
            tc.tile_pool(name="xtp", bufs=3) as xtp,
            tc.tile_pool(name="op", bufs=4) as op,
            tc.tile_pool(name="pso", bufs=8, space="PSUM") as pso,
        ):
            bias_sb = const.tile([P, DOUT_SH], f32)
            nc.sync.dma_start(
                bias_sb[:],
                b_d[:].rearrange("(a n) -> a n", a=1).to_broadcast((P, DOUT_SH)),
            )

            # resident transposed ternary weights per output group:
            #   bf16: wtg_bf[g][p, i, ko, j] = t[(c0+i)*128 + j, ko*128 + p]
            #   fp8 u16 pairs: wt16[g][p, b, i, j] = bytes
            #     (t[(c0+i)*128+j, KBF+256b+2p], t[..., KBF+256b+2p+1])
            wtg_bf = [
                wres.tile([P, cc, KO_BF, P], bf16, name=f"wtgbf_{g}")
                for g, (_, cc, _, _) in enumerate(GROUPS)
            ]
            wt16 = [
                wres.tile([P, NB8, cc, P], u16, name=f"wt16_{g}")
                for g, (_, cc, _, _) in enumerate(GROUPS)
            ]

            def emit_w_group(g):
                c0, cc, n0, width = GROUPS[g]
                for i in range(cc):
                    c = c0 + i
                    rc = min(P, DOUT_SH - c * P)  # 128 or 96 (last)
                    for q in range(NQ):
                        wrow = wq.tile([P, QCOL], f32, tag="wrow")
                        if rc < P:
                            nc.gpsimd.memset(wrow[:], 0.0)
                        # ACT ring: w-XBAR dispatches on the sync ring wait
                        # on DVE; loads here would queue behind those waits
                        nc.scalar.dma_start(
                            wrow[:rc, :],
                            w_d[c * P : c * P + rc, q * QCOL : (q + 1) * QCOL],
                        )
                        # t = (w >= a/2) - (w <= -a/2) in {-1,0,1}
                        le = wq.tile([P, QCOL], bf16, tag="le")
                        nc.any.tensor_scalar(le[:], wrow[:], -a2, None, Alu.is_le)
                        if q == 0:
                            tq = wq.tile([P, QCOL], bf16, tag="tq")
                            nc.vector.scalar_tensor_tensor(
                                tq[:], wrow[:], a2, le[:], Alu.is_ge, Alu.subtract
                            )
                            nc.sync.dma_start_transpose(
                                wtg_bf[g][:, i, :, :], tq[:]
                            )
                        else:
                            tq8 = wq.tile([P, QCOL], f8, tag="tq8")
                            nc.vector.scalar_tensor_tensor(
                                tq8[:], wrow[:], a2, le[:], Alu.is_ge, Alu.subtract
                            )
                            nc.sync.dma_start_transpose(
                                wt16[g][:, :, i, :], tq8[:].bitcast(u16)
                            )

            xt_blocks = {}

            def emit_prep(blk):
                xtb_bf = xtp.tile([P, MB, KO_BF, P], bf16, tag="xtbf")
                xtb8 = xtp.tile([P, MB, NB8, 2, P], f8, tag="xt8")
                for m in range(MB):
                    ms = blk * MB + m
                    xrow = xp.tile([P, DIN], f32, tag="xrow")
                    for h in range(4):
                        hw = DIN // 4
                        # ACT HWDGE ring: keeps the sync ring free for
                        # transposes so loads never queue behind their waits
                        nc.scalar.dma_start(
                            xrow[:, h * hw : (h + 1) * hw],
                            x_d[ms * P : (ms + 1) * P, h * hw : (h + 1) * hw],
                        )
                    xbf = xcp.tile([P, KBF], bf16, tag="xbf")
                    nc.vector.tensor_copy(xbf[:], xrow[:, :KBF])
                    x8 = xcp.tile([P, K8], f8, tag="x8")
                    nc.vector.tensor_copy(x8[:], xrow[:, KBF:])

                    nc.sync.dma_start_transpose(xtb_bf[:, m, :, :], xbf[:])
                    xt16 = xtq.tile([P, NB8, P], u16, tag="xt16")
                    nc.sync.dma_start_transpose(xt16[:], x8[:].bitcast(u16))
                    # de-interleave pairs into slot-block layout (ISA needs
                    # block layout for the stationary operand): 2 big copies
                    xtf8 = xt16[:].bitcast(f8).rearrange(
                        "p b (t two) -> p b two t", two=2
                    )
                    for sl in range(2):
                        nc.vector.tensor_copy(
                            xtb8[:, m, :, sl, :], xtf8[:, :, sl, :]
                        )
                xt_blocks[blk] = (xtb_bf, xtb8)

            def emit_compute_pass(blk, g):
                xtb_bf, xtb8 = xt_blocks[blk]
                if True:
                    c0, cc, n0, width = GROUPS[g]
                    pw = cc * P  # padded width (>= real width)
                    w8v = wt16[g][:].bitcast(f8).rearrange(
                        "p b c (d two) -> p b two (c d)", two=2
                    )
                    for m in range(MB):
                        ms = blk * MB + m
                        po = pso.tile([P, 512], f32, tag="po", name=f"po_{ms}_{g}")
                        for ko in range(KO_BF):
                            nc.tensor.matmul(
                                po[:, :pw],
                                xtb_bf[:, m, ko, :],
                                wtg_bf[g][:, :, ko, :],
                                start=(ko == 0),
                                stop=False,
                            )
                        for bb in range(NB8):
                            nc.tensor.matmul(
                                po[:, :pw],
                                xtb8[:, m, bb, :, :],
                                w8v[:, bb, :, :],
                                start=False,
                                stop=(bb == NB8 - 1),
                                perf_mode=DR,
                            )
                        # out = psum * alpha (ACT), then += bias (GpSimd)
                        osb = op.tile([P, 512], f32, tag="osb", name=f"osb_{ms}_{g}")
                        nc.scalar.activation(
                            osb[:, :width],
                            po[:, :width],
                            Act.Copy,
                            scale=float(alpha_f),
                        )
                        nc.gpsimd.tensor_add(
                            osb[:, :width],
                            osb[:, :width],
                            bias_sb[:, n0 : n0 + width],
                        )
                        # GpSimd SWDGE ring: follows the bias-add on the
                        # same queue; keeps both HWDGE rings unblocked
                        nc.gpsimd.dma_start(
                            o_d[ms * P : (ms + 1) * P, n0 : n0 + width],
                            osb[:, :width],
                        )

            def emit_compute(blk):
                for g in range(len(GROUPS)):
                    emit_compute_pass(blk, g)
                xt_blocks.pop(blk)

            # emission schedule: weight groups interleave with early preps.
            # The first 3 blocks are emitted group-major so the PE can run
            # g0-passes of several blocks while g1/g2 weights are still
            # being built, instead of head-of-line blocking on g2.
            emit_w_group(0)
            emit_prep(0)
            emit_w_group(1)
            emit_prep(1)
            emit_w_group(2)
            emit_prep(2)
            for g in range(len(GROUPS)):
                for blk in range(3):
                    emit_compute_pass(blk, g)
            for blk in range(3):
                xt_blocks.pop(blk)
            emit_prep(3)
            emit_prep(4)
            for blk in range(3, NBLK):
                emit_compute(blk)
                if blk + 2 < NBLK:
                    emit_prep(blk + 2)

    nc.compile()
    return nc


_CACHE = {}


def _get_nc(alpha_f):
    key = float(alpha_f)
    if key not in _CACHE:
        _CACHE[key] = _build(key)
    return _CACHE[key]


def kernel(x, w, alpha, bias):
    from concourse.bass_utils import run_bass_kernel_spmd

    alpha2 = np.ascontiguousarray(np.asarray(alpha, dtype=np.float32).reshape(1))
    nc = _get_nc(alpha2[0])
    x2 = np.ascontiguousarray(np.asarray(x, dtype=np.float32).reshape(TOK, DIN))
    in_maps = []
    for c in range(NCORES):
        in_maps.append(
            {
                "x": x2,
                "w": np.ascontiguousarray(w[c * DOUT_SH : (c + 1) * DOUT_SH]),
                "alpha": alpha2,
                "bias": np.ascontiguousarray(bias[c * DOUT_SH : (c + 1) * DOUT_SH]),
            }
        )
    res = run_bass_kernel_spmd(nc, in_maps, core_ids=list(range(NCORES)))
    outs = [res.results[c]["out"] for c in range(NCORES)]
    out = np.concatenate(outs, axis=1).reshape(B, S, DOUT)
    return np.ascontiguousarray(out.astype(np.float32))


# revision 14
# speedup vs baseline: 1.2887x; 1.2887x over previous
"""BitNet ternary linear (nn_BitNetLinear4Bit) Trainium2 Bass kernel.

out = x @ (alpha * clip(round(w/alpha), -1, 1))^T + bias
  x: [2, 2048, 4096] f32, w: [11008, 4096] f32, alpha: [1] f32, bias: [11008] f32
  -> out: [2, 2048, 11008] f32

Sharding: column-parallel over 8 cores. Each core gets the full x
(replicated) and a 1376-row slice of w / bias; it produces a
[4096, 1376] slice of the output which the host concatenates.

Per-core algorithm — mixed-precision K split to cut PE time:
  k < KBF (=2048): bf16 path. 16 normal bf16 matmuls per 128-token
    block per output group.
  k >= KBF: fp8e4 path at 2x rate via DoubleRow; w ternary values
    {-1,0,1} are exact in fp8. 8 DoubleRow matmuls of 256-k each.
    x is cast to e4m3 and XBAR-transposed as uint16 byte-pairs, then
    DVE de-interleaved into even/odd slot blocks (the ISA requires a
    block layout for the stationary operand). The moving (weight)
    operand reads the byte-interleaved pairs directly via stride-2
    APs.
  All transposes (x and w, both precisions) go through the DMA XBAR;
  the PE only runs matmuls. Work is emitted block-pipelined (MB=4
  token-blocks per block): weight-group prep and x-prep interleave
  so every strict-FIFO engine queue matches execution order, and
  compute for block b is emitted after prep for block b+... (2-block
  double buffering of transposed x tiles).
  Eviction: ACT copy*alpha, GpSimd +bias, out-DMA on the ACT HWDGE
  ring (keeps the sync ring free for loads/transposes).
  Expected rel err ~1.66e-2 (fp8 quantization of half of x),
  verified against an exact CPU model.

alpha is read on the host and baked into the program as an immediate;
the compiled program is cached keyed on alpha.
"""

import numpy as np

B, S, DIN, DOUT = 2, 2048, 4096, 11008
NCORES = 8
DOUT_SH = DOUT // NCORES  # 1376
TOK = B * S  # 4096
P = 128
KBF = 2048  # bf16 k range; the rest is fp8 DoubleRow
K8 = DIN - KBF
KO_BF = KBF // P  # 16
NB8 = K8 // (2 * P)  # 8 DoubleRow blocks of 256 k
MB = 2  # token-blocks per pipeline block
NBLK = TOK // P // MB  # 8


def _build(alpha_f, debug=False):
    import concourse.mybir as mybir
    from concourse import bacc
    from concourse.tile import TileContext

    f32 = mybir.dt.float32
    bf16 = mybir.dt.bfloat16
    f8 = mybir.dt.float8e4
    u16 = mybir.dt.uint16
    Alu = mybir.AluOpType
    Act = mybir.ActivationFunctionType
    DR = mybir.MatmulPerfMode.DoubleRow

    W_CHUNKS = (DOUT_SH + P - 1) // P  # 11 (last chunk 96 rows, zero-padded)
    QCOL = 2048
    NQ = DIN // QCOL  # 2 q-blocks; q=0 bf16, q=1 fp8
    GROUPS = []  # (first chunk, n chunks, dout start, real width)
    c = 0
    while c < W_CHUNKS:
        cc = min(4, W_CHUNKS - c)
        width = min(DOUT_SH, (c + cc) * P) - c * P
        GROUPS.append((c, cc, c * P, width))
        c += cc

    a2 = float(alpha_f) * 0.5

    nc = bacc.Bacc(None, target_bir_lowering=False, debug=debug)
    x_d = nc.dram_tensor("x", [TOK, DIN], f32, kind="ExternalInput")
    w_d = nc.dram_tensor("w", [DOUT_SH, DIN], f32, kind="ExternalInput")
    nc.dram_tensor("alpha", [1], f32, kind="ExternalInput")
    b_d = nc.dram_tensor("bias", [DOUT_SH], f32, kind="ExternalInput")
    o_d = nc.dram_tensor("out", [TOK, DOUT_SH], f32, kind="ExternalOutput")

    with TileContext(nc) as tc:
        with (
            tc.tile_pool(name="const", bufs=1) as const,
            tc.tile_pool(name="wres", bufs=1) as wres,
            tc.tile_pool(name="wq", bufs=2) as wq,
            tc.tile_pool(name="xp", bufs=2) as xp,
            tc.tile_pool(name="xcp", bufs=3) as xcp,
            tc.tile_pool(name="xtq", bufs=3) as xtq,
            tc.tile_pool(name="xtp", bufs=3) as xtp,
            tc.tile_pool(name="op", bufs=4) as op,
            tc.tile_pool(name="pso", bufs=8, space="PSUM") as pso,
        ):
            bias_sb = const.tile([P, DOUT_SH], f32)
            nc.sync.dma_start(
                bias_sb[:],
                b_d[:].rearrange("(a n) -> a n", a=1).to_broadcast((P, DOUT_SH)),
            )

            # resident transposed ternary weights per output group:
            #   bf16: wtg_bf[g][p, i, ko, j] = t[(c0+i)*128 + j, ko*128 + p]
            #   fp8 u16 pairs: wt16[g][p, b, i, j] = bytes
            #     (t[(c0+i)*128+j, KBF+256b+2p], t[..., KBF+256b+2p+1])
            wtg_bf = [
                wres.tile([P, cc, KO_BF, P], bf16, name=f"wtgbf_{g}")
                for g, (_, cc, _, _) in enumerate(GROUPS)
            ]
            wt16 = [
                wres.tile([P, NB8, cc, P], u16, name=f"wt16_{g}")
                for g, (_, cc, _, _) in enumerate(GROUPS)
            ]

            def emit_w_group(g):
                c0, cc, n0, width = GROUPS[g]
                for i in range(cc):
                    c = c0 + i
                    rc = min(P, DOUT_SH - c * P)  # 128 or 96 (last)
                    for q in range(NQ):
                        wrow = wq.tile([P, QCOL], f32, tag="wrow")
                        if rc < P:
                            nc.gpsimd.memset(wrow[:], 0.0)
                        nc.sync.dma_start(
                            wrow[:rc, :],
                            w_d[c * P : c * P + rc, q * QCOL : (q + 1) * QCOL],
                        )
                        # t = (w >= a/2) - (w <= -a/2) in {-1,0,1}
                        le = wq.tile([P, QCOL], bf16, tag="le")
                        nc.any.tensor_scalar(le[:], wrow[:], -a2, None, Alu.is_le)
                        if q == 0:
                            tq = wq.tile([P, QCOL], bf16, tag="tq")
                            nc.vector.scalar_tensor_tensor(
                                tq[:], wrow[:], a2, le[:], Alu.is_ge, Alu.subtract
                            )
                            nc.sync.dma_start_transpose(
                                wtg_bf[g][:, i, :, :], tq[:]
                            )
                        else:
                            tq8 = wq.tile([P, QCOL], f8, tag="tq8")
                            nc.vector.scalar_tensor_tensor(
                                tq8[:], wrow[:], a2, le[:], Alu.is_ge, Alu.subtract
                            )
                            nc.sync.dma_start_transpose(
                                wt16[g][:, :, i, :], tq8[:].bitcast(u16)
                            )

            xt_blocks = {}

            def emit_prep(blk):
                xtb_bf = xtp.tile([P, MB, KO_BF, P], bf16, tag="xtbf")
                xtb8 = xtp.tile([P, MB, NB8, 2, P], f8, tag="xt8")
                for m in range(MB):
                    ms = blk * MB + m
                    xrow = xp.tile([P, DIN], f32, tag="xrow")
                    # one DMA per token-block: 16KB contiguous per partition,
                    # single dispatch on the ACT ring
                    nc.scalar.dma_start(xrow[:], x_d[ms * P : (ms + 1) * P, :])
                    xbf = xcp.tile([P, KBF], bf16, tag="xbf")
                    nc.vector.tensor_copy(xbf[:], xrow[:, :KBF])
                    x8 = xcp.tile([P, K8], f8, tag="x8")
                    nc.vector.tensor_copy(x8[:], xrow[:, KBF:])

                    nc.sync.dma_start_transpose(xtb_bf[:, m, :, :], xbf[:])
                    xt16 = xtq.tile([P, NB8, P], u16, tag="xt16")
                    nc.sync.dma_start_transpose(xt16[:], x8[:].bitcast(u16))
                    # de-interleave pairs into slot-block layout (ISA needs
                    # block layout for the stationary operand): 2 big copies
                    xtf8 = xt16[:].bitcast(f8).rearrange(
                        "p b (t two) -> p b two t", two=2
                    )
                    for sl in range(2):
                        nc.vector.tensor_copy(
                            xtb8[:, m, :, sl, :], xtf8[:, :, sl, :]
                        )
                xt_blocks[blk] = (xtb_bf, xtb8)

            def emit_compute_pass(blk, g):
                xtb_bf, xtb8 = xt_blocks[blk]
                if True:
                    c0, cc, n0, width = GROUPS[g]
                    pw = cc * P  # padded width (>= real width)
                    w8v = wt16[g][:].bitcast(f8).rearrange(
                        "p b c (d two) -> p b two (c d)", two=2
                    )
                    for m in range(MB):
                        ms = blk * MB + m
                        po = pso.tile([P, 512], f32, tag="po", name=f"po_{ms}_{g}")
                        for ko in range(KO_BF):
                            nc.tensor.matmul(
                                po[:, :pw],
                                xtb_bf[:, m, ko, :],
                                wtg_bf[g][:, :, ko, :],
                                start=(ko == 0),
                                stop=False,
                            )
                        for bb in range(NB8):
                            nc.tensor.matmul(
                                po[:, :pw],
                                xtb8[:, m, bb, :, :],
                                w8v[:, bb, :, :],
                                start=False,
                                stop=(bb == NB8 - 1),
                                perf_mode=DR,
                            )
                        # out = psum * alpha (ACT), then += bias (GpSimd)
                        osb = op.tile([P, 512], f32, tag="osb", name=f"osb_{ms}_{g}")
                        nc.scalar.activation(
                            osb[:, :width],
                            po[:, :width],
                            Act.Copy,
                            scale=float(alpha_f),
                        )
                        nc.gpsimd.tensor_add(
                            osb[:, :width],
                            osb[:, :width],
                            bias_sb[:, n0 : n0 + width],
                        )
                        # GpSimd SWDGE ring: follows the bias-add on the
                        # same queue; keeps both HWDGE rings unblocked
                        nc.gpsimd.dma_start(
                            o_d[ms * P : (ms + 1) * P, n0 : n0 + width],
                            osb[:, :width],
                        )

            def emit_compute(blk):
                for g in range(len(GROUPS)):
                    emit_compute_pass(blk, g)
                xt_blocks.pop(blk)

            # emission schedule: weight groups interleave with early preps.
            # The first 3 blocks are emitted group-major so the PE can run
            # g0-passes of several blocks while g1/g2 weights are still
            # being built, instead of head-of-line blocking on g2.
            emit_w_group(0)
            emit_prep(0)
            emit_w_group(1)
            emit_prep(1)
            emit_w_group(2)
            emit_prep(2)
            for g in range(len(GROUPS)):
                for blk in range(3):
                    emit_compute_pass(blk, g)
            for blk in range(3):
                xt_blocks.pop(blk)
            emit_prep(3)
            emit_prep(4)
            for blk in range(3, NBLK):
                emit_compute(blk)
                if blk + 2 < NBLK:
                    emit_prep(blk + 2)

    nc.compile()
    return nc


_CACHE = {}


def _get_nc(alpha_f):
    key = float(alpha_f)
    if key not in _CACHE:
        _CACHE[key] = _build(key)
    return _CACHE[key]


def kernel(x, w, alpha, bias):
    from concourse.bass_utils import run_bass_kernel_spmd

    alpha2 = np.ascontiguousarray(np.asarray(alpha, dtype=np.float32).reshape(1))
    nc = _get_nc(alpha2[0])
    x2 = np.ascontiguousarray(np.asarray(x, dtype=np.float32).reshape(TOK, DIN))
    in_maps = []
    for c in range(NCORES):
        in_maps.append(
            {
                "x": x2,
                "w": np.ascontiguousarray(w[c * DOUT_SH : (c + 1) * DOUT_SH]),
                "alpha": alpha2,
                "bias": np.ascontiguousarray(bias[c * DOUT_SH : (c + 1) * DOUT_SH]),
            }
        )
    res = run_bass_kernel_spmd(nc, in_maps, core_ids=list(range(NCORES)))
    outs = [res.results[c]["out"] for c in range(NCORES)]
    out = np.concatenate(outs, axis=1).reshape(B, S, DOUT)
    return np.ascontiguousarray(out.astype(np.float32))
